# revision 34
# baseline (speedup 1.0000x reference)
"""Trainium2 Bass kernel for the GWFFN spiking-CNN block (nn_GWFFN).

Reference computation (multi-step LIF scan over T=4, eval-mode BN):
  up:   LIF -> 1x1 conv (128->512) -> BN
  conv: LIF -> grouped 3x3 conv (512->512, groups=8, pad=1) -> BN -> +h1
  down: LIF -> 1x1 conv (512->128) -> BN -> +x

Sharding: data-parallel over batch N=16 -> 8 cores x 2 samples. Weights are
replicated, no collectives; each core runs an identical program on its slice.

Per-core layout: channels on SBUF partitions (the 512-wide inner dim is 4
tiles of 128), free dim = (n_loc=2, h=32, w=32) = 2048 positions, one
time-step at a time. Scaling the LIF state by 2 (v_t = w_t/2, u' = v after
reset) turns the scan into
    w_t = u'_{t-1} + x_t ;  s_t = (w_t >= 2) ;  u'_t = w_t * m_t
with m_t = 0.5*(w_t < 2). The matmuls consume m instead of s via
s = 1 - 2m: weights are scaled by -2 and the constant W@1 term folds into a
per-channel bias (applied for free in the ACT-engine PSUM->SBUF evacuation;
the grouped conv's zero-pad border stores m=0.5 so the ones-field is exact
at the edges too). This keeps every elementwise op on the fast
tensor_tensor / dual-op tensor_scalar paths (the TensorScalarPtr and
Pool-engine comparison ops measure 10-30x slower).

Matmuls run in bf16 (m is exactly {0, 0.5} in bf16; BN scale folded into
weights on the host). The grouped 3x3 conv is 9 shifted 1x1 matmuls
accumulating in PSUM over the padded [2,34,36] layout, packed as 4
concurrent 64x64 tile_position matmuls (4 groups at once); the +h1 residual
is preloaded into PSUM with identity / half-swap permutation matmuls (the
anti-diagonal array tiles emit their groups partition-swapped; the swap is
undone by permuting the down-projection weight rows on the host). The
down-projection bias is added with a K=2 rank-2 matmul of a bf16 hi/lo
split of the bias against a ones vector.
"""

import numpy as np
import ml_dtypes

import concourse.bacc as bacc
import concourse.mybir as mybir
import concourse.tile as tile
from concourse.bass_utils import run_bass_kernel_spmd

FP32 = mybir.dt.float32
BF16 = mybir.dt.bfloat16
ALU = mybir.AluOpType
ACTF = mybir.ActivationFunctionType
BF = ml_dtypes.bfloat16

T, NFULL, C, H, W = 4, 16, 128, 32, 32
INNER, GROUPS = 512, 8
NCORES = 8
NLOC = NFULL // NCORES  # 2
HW = H * W              # 1024
F = NLOC * HW           # 2048 free positions per time-step
CH = 512                # matmul free-dim chunk (one PSUM bank fp32)
HP, WP = H + 2, W + 4   # padded spatial (W padded by 2 each side: 4B align)
EPS = 1e-5

_CACHE = {}


def _mm(nc, out, lhsT, rhs, start, stop):
    nc.tensor.matmul(out, lhsT, rhs, start=start, stop=stop,
                     skip_group_check=True)


def _build_nc():
    nc = bacc.Bacc("TRN2", target_bir_lowering=False)

    x_d = nc.dram_tensor("x", [T, NLOC, C, H, W], FP32, kind="ExternalInput")
    wup_d = nc.dram_tensor("wupT", [C, INNER], BF16, kind="ExternalInput")
    wcv_d = nc.dram_tensor("wconvP", [128, 2, 9, 128], BF16, kind="ExternalInput")
    wdn_d = nc.dram_tensor("wdnT", [128, 4, 128], BF16, kind="ExternalInput")
    jm_d = nc.dram_tensor("jmat", [128, 8, 128], BF16, kind="ExternalInput")
    bia_d = nc.dram_tensor("bias", [128, 36], FP32, kind="ExternalInput")
    bdn_d = nc.dram_tensor("biasdn2", [2, 128], BF16, kind="ExternalInput")
    o_d = nc.dram_tensor("out", [T, NLOC, C, H, W], FP32, kind="ExternalOutput")

    with tile.TileContext(nc) as tc:
        with (
            tc.tile_pool(name="const", bufs=1) as cpool,
            tc.tile_pool(name="state", bufs=1) as spool,
            tc.tile_pool(name="work", bufs=2) as wpool,
            tc.tile_pool(name="psum", bufs=4, space="PSUM") as ppool,
        ):
            # ---- constants -------------------------------------------------
            wup_sb = cpool.tile([C, INNER], BF16)
            nc.sync.dma_start(out=wup_sb[:], in_=wup_d[:])
            wcv_sb = cpool.tile([128, 2, 9, 128], BF16)
            nc.sync.dma_start(out=wcv_sb[:], in_=wcv_d[:])
            wdn_sb = cpool.tile([128, 4, 128], BF16)
            nc.sync.dma_start(out=wdn_sb[:], in_=wdn_d[:])
            jm_sb = cpool.tile([128, 8, 128], BF16)
            nc.sync.dma_start(out=jm_sb[:], in_=jm_d[:])
            bia_sb = cpool.tile([128, 36], FP32)
            nc.sync.dma_start(out=bia_sb[:], in_=bia_d[:])
            bdn_sb = cpool.tile([2, 128], BF16)
            nc.sync.dma_start(out=bdn_sb[:], in_=bdn_d[:])
            ones_sb = cpool.tile([2, CH], BF16)
            nc.gpsimd.memset(ones_sb[:], 1.0)

            # ---- persistent LIF state + padded m2 buffers ------------------
            u1 = spool.tile([128, F], BF16)
            u2, u3 = [], []
            s2pp = [[], []]
            for i in range(4):
                # u tiles need no memset: at t=0 every consumer reads the
                # aliased h/x tiles instead, and the first u write is a
                # full-tile overwrite
                u2t = spool.tile([128, F], BF16, name=f"u2_{i}")
                u2.append(u2t)
                u3t = spool.tile([128, F], BF16, name=f"u3_{i}")
                u3.append(u3t)
                for par in range(2):
                    # double-buffered by t parity so m2(t) never waits on
                    # conv(t-1) tap reads; border 1.0 == "no spike"
                    s2t = spool.tile([128, NLOC, HP, WP], BF16,
                                     name=f"s2p_{par}_{i}")
                    (nc.vector if i % 2 == 0 else nc.gpsimd).memset(
                        s2t[:], 1.0)
                    s2pp[par].append(s2t)

            # down-stage of step t-1, emitted after conv(t) so the PE
            # stream never waits on the t-1 LIF3 chain
            pend = None

            def emit_down(t, m3, x_sb):
                out_sb = wpool.tile([128, F], FP32, tag="osb", bufs=2,
                                    name=f"osb_{t}")
                ps_dn = [ppool.tile([128, 2 * CH], FP32, tag="ps",
                                    name=f"psdn_{t}_{p}") for p in range(2)]
                for kt in range(4):       # kt-outer: 4 MMs reuse one LDW
                    for p in range(2):
                        for hh in range(2):
                            c = 2 * p + hh
                            _mm(nc, ps_dn[p][:, hh * CH:(hh + 1) * CH],
                                wdn_sb[:, kt, :],
                                m3[kt][:, c * CH:(c + 1) * CH],
                                start=(kt == 0), stop=False)
                for p in range(2):
                    for hh in range(2):
                        # + bias_dn via rank-2 ones matmul (bf16 hi+lo)
                        _mm(nc, ps_dn[p][:, hh * CH:(hh + 1) * CH],
                            bdn_sb[:], ones_sb[:], start=False, stop=True)
                    nc.vector.tensor_tensor(
                        out=out_sb[:, p * HW:(p + 1) * HW],
                        in0=ps_dn[p][:],
                        in1=x_sb[:, p * HW:(p + 1) * HW],
                        op=ALU.add)
                nc.sync.dma_start(
                    out=o_d[t].rearrange("n c h w -> c n h w"),
                    in_=out_sb.rearrange("p (n h w) -> p n h w", n=NLOC, h=H))

            for t in range(T):
                s2p = s2pp[t % 2]
                # ---- load x_t --------------------------------------------
                x_sb = wpool.tile([128, F], FP32, tag="x", bufs=3,
                                  name=f"x_{t}")
                nc.sync.dma_start(
                    out=x_sb.rearrange("p (n h w) -> p n h w", n=NLOC, h=H),
                    in_=x_d[t].rearrange("n c h w -> c n h w"))

                # ---- LIF1 (bf16, 2^t-scaled like LIF2/3) -----------------
                xs = wpool.tile([128, F], BF16, tag="xs", bufs=1,
                                name=f"xs_{t}")
                nc.vector.tensor_scalar(
                    out=xs[:], in0=x_sb[:], scalar1=float(2 ** t),
                    scalar2=None, op0=ALU.mult)
                if t == 0:
                    w1 = xs
                else:
                    w1 = wpool.tile([128, F], BF16, tag="w1", bufs=1,
                                    name=f"w1_{t}")
                    nc.vector.tensor_tensor(
                        out=w1[:], in0=u1[:], in1=xs[:], op=ALU.add)
                m1 = wpool.tile([128, F], BF16, tag="m1", bufs=1,
                                name=f"m1_{t}")
                nc.vector.tensor_scalar(
                    out=m1[:], in0=w1[:], scalar1=float(2 ** (t + 1)),
                    scalar2=None, op0=ALU.is_lt)
                nc.vector.tensor_tensor(
                    out=u1[:], in0=w1[:], in1=m1[:], op=ALU.mult)

                # ---- down stage of step t-1 (after LIF1 so m1(t) is ready
                # before the PE reaches up(t)) -----------------------------
                if pend is not None:
                    emit_down(*pend)

                # ---- up 1x1 matmul + evac(+bias) + LIF2 ------------------
                h1, w2 = [], []
                for ct in range(4):
                    h1t = wpool.tile([128, F], BF16, tag="hbuf", bufs=6,
                                     name=f"h1_{t}_{ct}")
                    for p in range(2):
                        ps_up = ppool.tile([128, 2 * CH], FP32, tag="ps",
                                           name=f"psup_{t}_{ct}_{p}")
                        for hh in range(2):
                            c = 2 * p + hh
                            _mm(nc, ps_up[:, hh * CH:(hh + 1) * CH],
                                wup_sb[:, 128 * ct:128 * (ct + 1)],
                                m1[:, c * CH:(c + 1) * CH],
                                start=True, stop=True)
                        nc.scalar.activation(
                            out=h1t[:, p * HW:(p + 1) * HW], in_=ps_up[:],
                            func=ACTF.Identity,
                            bias=bia_sb[:, 8 * t + ct:8 * t + ct + 1],
                            scale=float(2 ** t))
                    h1.append(h1t)

                    if t == 0:
                        w2t = h1t
                    else:
                        w2t = wpool.tile([128, F], BF16, tag="wbuf", bufs=5,
                                         name=f"w2_{t}_{ct}")
                        nc.vector.tensor_tensor(
                            out=w2t[:], in0=u2[ct][:], in1=h1t[:], op=ALU.add)
                    w2.append(w2t)
                    nc.vector.tensor_scalar(
                        out=s2p[ct][:, :, 1:1 + H, 2:2 + W],
                        in0=w2t.rearrange("p (n h w) -> p n h w", n=NLOC, h=H),
                        scalar1=float(2 ** (t + 1)), scalar2=None,
                        op0=ALU.is_lt)
                    ueng = nc.vector if ct < 2 else nc.gpsimd
                    ueng.tensor_tensor(
                        out=u2[ct].rearrange("p (n h w) -> p n h w",
                                             n=NLOC, h=H),
                        in0=w2t.rearrange("p (n h w) -> p n h w", n=NLOC, h=H),
                        in1=s2p[ct][:, :, 1:1 + H, 2:2 + W],
                        op=ALU.mult)

                # ---- grouped 3x3 conv (+h1 preload) + evac(+bias) --------
                h2 = [None] * 4
                for q in range(2):
                    ta, tb = 2 * q, 2 * q + 1   # s2 tiles feeding this quad
                    h2a = wpool.tile([128, F], BF16, tag="hbuf", bufs=6,
                                     name=f"h2_{t}_{ta}")
                    h2b = wpool.tile([128, F], BF16, tag="hbuf", bufs=6,
                                     name=f"h2_{t}_{tb}")
                    h2[ta], h2[tb] = h2a, h2b
                    for p in range(2):
                        P1 = ppool.tile([128, 2 * CH], FP32, tag="ps",
                                        name=f"psc1_{t}_{q}_{p}")
                        P2 = ppool.tile([128, 2 * CH], FP32, tag="ps",
                                        name=f"psc2_{t}_{q}_{p}")
                        for hh in range(2):
                            c = 2 * p + hh
                            sl = slice(c * CH, (c + 1) * CH)
                            _mm(nc, P1[:, hh * CH:(hh + 1) * CH],
                                jm_sb[:, 2 * t, :], h1[ta][:, sl],
                                start=True, stop=False)
                        for hh in range(2):
                            c = 2 * p + hh
                            sl = slice(c * CH, (c + 1) * CH)
                            _mm(nc, P2[:, hh * CH:(hh + 1) * CH],
                                jm_sb[:, 2 * t + 1, :], h1[tb][:, sl],
                                start=True, stop=False)
                        for tap in range(9):
                            dy, dx = tap // 3, tap % 3
                            last = tap == 8
                            wq = wcv_sb[:, q, tap, :]
                            for hh in range(2):
                                h0 = 16 * hh
                                osl = slice(hh * CH, (hh + 1) * CH)
                                ra = s2p[ta][:, p, h0 + dy:h0 + dy + 16,
                                             1 + dx:33 + dx]
                                rb = s2p[tb][:, p, h0 + dy:h0 + dy + 16,
                                             1 + dx:33 + dx]
                                # T1: group 4q   rows 0-63  -> P1[0:64]
                                _mm(nc, P1[0:64, osl], wq[0:64, 0:64],
                                    ra[0:64], start=False, stop=last)
                                # T2: group 4q+1 rows 64-127 -> P1[64:128]
                                _mm(nc, P1[64:128, osl], wq[64:128, 64:128],
                                    ra[64:128], start=False, stop=last)
                                # T3: group 4q+2 rows 0-63  -> P2[64:128]
                                _mm(nc, P2[64:128, osl], wq[0:64, 64:128],
                                    rb[0:64], start=False, stop=last)
                                # T4: group 4q+3 rows 64-127 -> P2[0:64]
                                _mm(nc, P2[0:64, osl], wq[64:128, 0:64],
                                    rb[64:128], start=False, stop=last)
                        nc.scalar.activation(
                            out=h2a[:, p * HW:(p + 1) * HW], in_=P1[:],
                            func=ACTF.Identity,
                            bias=bia_sb[:, 8 * t + 4 + ta:8 * t + 5 + ta],
                            scale=float(2 ** t))
                        nc.scalar.activation(
                            out=h2b[:, p * HW:(p + 1) * HW], in_=P2[:],
                            func=ACTF.Identity,
                            bias=bia_sb[:, 8 * t + 4 + tb:8 * t + 5 + tb],
                            scale=float(2 ** t))

                # ---- LIF3 ------------------------------------------------
                m3 = []
                for ct in range(4):
                    if t == 0:
                        w3t = h2[ct]
                    else:
                        w3t = wpool.tile([128, F], BF16, tag="wbuf", bufs=5,
                                         name=f"w3_{t}_{ct}")
                        nc.vector.tensor_tensor(
                            out=w3t[:], in0=u3[ct][:], in1=h2[ct][:],
                            op=ALU.add)
                    m3t = wpool.tile([128, F], BF16, tag="m3", bufs=6,
                                     name=f"m3_{t}_{ct}")
                    nc.vector.tensor_scalar(
                        out=m3t[:], in0=w3t[:],
                        scalar1=float(2 ** (t + 1)), scalar2=None,
                        op0=ALU.is_lt)
                    m3.append(m3t)
                    u3eng = nc.vector if ct < 2 else nc.gpsimd
                    u3eng.tensor_tensor(
                        out=u3[ct][:], in0=w3t[:], in1=m3t[:], op=ALU.mult)

                pend = (t, m3, x_sb)

            emit_down(*pend)

    nc.compile()
    return nc


def _prep_weights(inputs):
    """Fold BN into weights, apply the s = 1-2m encoding (scale by -2 and
    compute per-channel ones-biases), pack/permute for the on-chip layout."""
    f32 = np.float32
    sc_up = (inputs["g_up"] / np.sqrt(inputs["v_up"] + EPS)).astype(f32)
    sc_cv = (inputs["g_conv"] / np.sqrt(inputs["v_conv"] + EPS)).astype(f32)
    sc_dn = (inputs["g_down"] / np.sqrt(inputs["v_down"] + EPS)).astype(f32)
    shifts = []
    for nm, sc in (("up", sc_up), ("conv", sc_cv), ("down", sc_dn)):
        shifts.append(inputs[f"b_{nm}"] - inputs[f"m_{nm}"] * sc)
    if max(np.abs(s).max() for s in shifts) > 0:
        raise NotImplementedError("nonzero BN shift not supported")

    w_up = np.asarray(inputs["w_up"], f32)[:, :, 0, 0] * sc_up[:, None]
    wupT = np.ascontiguousarray((-1.0 * w_up).T).astype(BF)    # [128, 512]
    # exact negative sum of the *rounded* weights: the dense m=1
    # background then cancels exactly and only spike terms carry bf16 error
    bias_up = -1.0 * wupT.astype(np.float64).sum(axis=0)       # [512]

    w_cv = np.asarray(inputs["w_conv"], f32) * sc_cv[:, None, None, None]
    wcvP = np.zeros((128, 2, 9, 128), f32)
    for q in range(2):
        for tap in range(9):
            dy, dx = tap // 3, tap % 3

            def blk(g):
                # W_g[ci, co] = -w_conv_eff[64g + co, ci, dy, dx]
                return np.ascontiguousarray(
                    -1.0 * w_cv[64 * g:64 * (g + 1), :, dy, dx].T)
            wcvP[0:64, q, tap, 0:64] = blk(4 * q)
            wcvP[64:128, q, tap, 64:128] = blk(4 * q + 1)
            wcvP[0:64, q, tap, 64:128] = blk(4 * q + 2)
            wcvP[64:128, q, tap, 0:64] = blk(4 * q + 3)
    wcvP = wcvP.astype(BF)
    # conv biases directly per psum partition m (P1 diag / P2 anti-diag),
    # again as exact negative half-sums of the rounded packed weights
    w64 = wcvP.astype(np.float64)
    bias_cv = np.zeros((4, 128))
    for q in range(2):
        lo = w64[0:64, q].sum(axis=(0, 1))     # [128] sum over rows<64, taps
        hi = w64[64:128, q].sum(axis=(0, 1))   # [128] sum over rows>=64
        bias_cv[2 * q] = -1.0 * np.concatenate([lo[:64], hi[64:]])
        bias_cv[2 * q + 1] = -1.0 * np.concatenate([hi[:64], lo[64:]])

    w_dn = np.asarray(inputs["w_down"], f32)[:, :, 0, 0] * sc_dn[:, None]
    # s3/m3 tile layouts: kt even natural, kt odd half-swapped ([g3|g2]...)
    wdnT = np.zeros((128, 4, 128), f32)
    for kt in range(4):
        rows = np.arange(128) + 128 * kt
        if kt % 2 == 1:
            rows = np.concatenate([rows[64:], rows[:64]])
        wdnT[:, kt, :] = -1.0 * w_dn[:, rows].T
    wdnT = wdnT.astype(BF)
    bias_dn = -1.0 * wdnT.astype(np.float64).sum(axis=(0, 1))  # [128]

    # per-t preload matrices: 2^-(t+1) * identity / half-swap (the h1 tiles
    # hold 2^(t+1)-scaled values; the preload rescales them back)
    jm = np.zeros((128, 8, 128), f32)
    for t in range(4):
        sc = 2.0 ** -t
        jm[np.arange(128), 2 * t, np.arange(128)] = sc
        jm[np.arange(128), 2 * t + 1, (np.arange(128) + 64) % 128] = sc
    jm = jm.astype(BF)

    # bias tile [128, 8]: cols 0-3 = up bias per tile; 4-7 = conv bias per
    # conv-out tile (odd tiles half-swapped to match the P2 psum layout)
    # per-t scaled bias tile [128, 8*t + slot]; ACT computes
    # 2^(t+1)*psum + bias so the bias columns carry the same scale
    bias = np.zeros((128, 36), f32)
    for t in range(4):
        bias[:, 32 + t] = 2.0 ** 30 * 2.0 ** (t + 1)
        sc = 2.0 ** t
        for ct in range(4):
            bias[:, 8 * t + ct] = sc * bias_up[128 * ct:128 * (ct + 1)]
            bias[:, 8 * t + 4 + ct] = sc * bias_cv[ct]
    # down bias as bf16 hi + lo rows against a ones vector
    bdn_hi = bias_dn.astype(BF)
    bdn_lo = (bias_dn - bdn_hi.astype(f32)).astype(BF)
    bdn2 = np.stack([bdn_hi, bdn_lo], axis=0)                  # [2, 128]

    return wupT, wcvP, wdnT, jm, bias, bdn2


def run(inputs, trace=False):
    if "nc" not in _CACHE:
        _CACHE["nc"] = _build_nc()
    nc = _CACHE["nc"]

    wupT, wcvP, wdnT, jm, bias, bdn2 = _prep_weights(inputs)
    x = np.asarray(inputs["x"], np.float32)
    in_maps = []
    for i in range(NCORES):
        in_maps.append({
            "x": np.ascontiguousarray(x[:, NLOC * i:NLOC * (i + 1)]),
            "wupT": wupT, "wconvP": wcvP, "wdnT": wdnT, "jmat": jm,
            "bias": bias, "biasdn2": bdn2,
        })
    res = run_bass_kernel_spmd(nc, in_maps, core_ids=list(range(NCORES)),
                               trace=trace)
    out = np.concatenate([r["out"] for r in res.results], axis=1)
    return out, res


def kernel(**inputs):
    out, _ = run(inputs, trace=False)
    return out


# revision 35
# speedup vs baseline: 1.1645x; 1.1645x over previous
"""Trainium2 Bass kernel for the GWFFN spiking-CNN block (nn_GWFFN).

Reference computation (multi-step LIF scan over T=4, eval-mode BN):
  up:   LIF -> 1x1 conv (128->512) -> BN
  conv: LIF -> grouped 3x3 conv (512->512, groups=8, pad=1) -> BN -> +h1
  down: LIF -> 1x1 conv (512->128) -> BN -> +x

Sharding: data-parallel over batch N=16 -> 8 cores x 2 samples. Weights are
replicated, no collectives; each core runs an identical program on its slice.

Per-core layout: channels on SBUF partitions (the 512-wide inner dim is 4
tiles of 128), free dim = (n_loc=2, h=32, w=32) = 2048 positions, one
time-step at a time. Scaling the LIF state by 2 (v_t = w_t/2, u' = v after
reset) turns the scan into
    w_t = u'_{t-1} + x_t ;  s_t = (w_t >= 2) ;  u'_t = w_t * m_t
with m_t = 0.5*(w_t < 2). The matmuls consume m instead of s via
s = 1 - 2m: weights are scaled by -2 and the constant W@1 term folds into a
per-channel bias (applied for free in the ACT-engine PSUM->SBUF evacuation;
the grouped conv's zero-pad border stores m=0.5 so the ones-field is exact
at the edges too). This keeps every elementwise op on the fast
tensor_tensor / dual-op tensor_scalar paths (the TensorScalarPtr and
Pool-engine comparison ops measure 10-30x slower).

Matmuls run in bf16 (m is exactly {0, 0.5} in bf16; BN scale folded into
weights on the host). The grouped 3x3 conv is 9 shifted 1x1 matmuls
accumulating in PSUM over the padded [2,34,36] layout, packed as 4
concurrent 64x64 tile_position matmuls (4 groups at once); the +h1 residual
is preloaded into PSUM with identity / half-swap permutation matmuls (the
anti-diagonal array tiles emit their groups partition-swapped; the swap is
undone by permuting the down-projection weight rows on the host). The
down-projection bias is added with a K=2 rank-2 matmul of a bf16 hi/lo
split of the bias against a ones vector.
"""

import numpy as np
import ml_dtypes

import concourse.bacc as bacc
import concourse.mybir as mybir
import concourse.tile as tile
from concourse.bass_utils import run_bass_kernel_spmd

FP32 = mybir.dt.float32
BF16 = mybir.dt.bfloat16
ALU = mybir.AluOpType
ACTF = mybir.ActivationFunctionType
BF = ml_dtypes.bfloat16

T, NFULL, C, H, W = 4, 16, 128, 32, 32
INNER, GROUPS = 512, 8
NCORES = 8
NLOC = NFULL // NCORES  # 2
HW = H * W              # 1024
F = NLOC * HW           # 2048 free positions per time-step
CH = 512                # matmul free-dim chunk (one PSUM bank fp32)
HP, WP = H + 2, W + 4   # padded spatial (W padded by 2 each side: 4B align)
EPS = 1e-5

_CACHE = {}


def _mm(nc, out, lhsT, rhs, start, stop):
    nc.tensor.matmul(out, lhsT, rhs, start=start, stop=stop,
                     skip_group_check=True)


def _build_nc():
    nc = bacc.Bacc("TRN2", target_bir_lowering=False)

    x_d = nc.dram_tensor("x", [T, NLOC, C, H, W], FP32, kind="ExternalInput")
    wup_d = nc.dram_tensor("wupT", [C, INNER], BF16, kind="ExternalInput")
    wcv_d = nc.dram_tensor("wconvP", [128, 2, 9, 128], BF16, kind="ExternalInput")
    wdn_d = nc.dram_tensor("wdnT", [128, 4, 128], BF16, kind="ExternalInput")
    jm_d = nc.dram_tensor("jmat", [128, 8, 128], BF16, kind="ExternalInput")
    bia_d = nc.dram_tensor("bias", [128, 36], FP32, kind="ExternalInput")
    bdn_d = nc.dram_tensor("biasdn2", [2, 128], BF16, kind="ExternalInput")
    o_d = nc.dram_tensor("out", [T, NLOC, C, H, W], FP32, kind="ExternalOutput")

    with tile.TileContext(nc) as tc:
        with (
            tc.tile_pool(name="const", bufs=1) as cpool,
            tc.tile_pool(name="state", bufs=1) as spool,
            tc.tile_pool(name="work", bufs=2) as wpool,
            tc.tile_pool(name="psum", bufs=4, space="PSUM") as ppool,
        ):
            # ---- constants -------------------------------------------------
            wup_sb = cpool.tile([C, INNER], BF16)
            nc.sync.dma_start(out=wup_sb[:], in_=wup_d[:])
            wcv_sb = cpool.tile([128, 2, 9, 128], BF16)
            nc.sync.dma_start(out=wcv_sb[:], in_=wcv_d[:])
            wdn_sb = cpool.tile([128, 4, 128], BF16)
            nc.sync.dma_start(out=wdn_sb[:], in_=wdn_d[:])
            jm_sb = cpool.tile([128, 8, 128], BF16)
            nc.sync.dma_start(out=jm_sb[:], in_=jm_d[:])
            bia_sb = cpool.tile([128, 36], FP32)
            nc.sync.dma_start(out=bia_sb[:], in_=bia_d[:])
            bdn_sb = cpool.tile([2, 128], BF16)
            nc.sync.dma_start(out=bdn_sb[:], in_=bdn_d[:])
            ones_sb = cpool.tile([2, CH], BF16)
            nc.gpsimd.memset(ones_sb[:], 1.0)

            # ---- persistent LIF state + padded m2 buffers ------------------
            u1 = spool.tile([128, F], BF16)
            u2, u3 = [], []
            s2pp = [[], []]
            for i in range(4):
                # u tiles need no memset: at t=0 every consumer reads the
                # aliased h/x tiles instead, and the first u write is a
                # full-tile overwrite
                u2t = spool.tile([128, F], BF16, name=f"u2_{i}")
                u2.append(u2t)
                u3t = spool.tile([128, F], BF16, name=f"u3_{i}")
                u3.append(u3t)
                for par in range(2):
                    # double-buffered by t parity so m2(t) never waits on
                    # conv(t-1) tap reads; border 1.0 == "no spike"
                    s2t = spool.tile([128, NLOC, HP, WP], BF16,
                                     name=f"s2p_{par}_{i}")
                    (nc.vector if i % 2 == 0 else nc.gpsimd).memset(
                        s2t[:], 1.0)
                    s2pp[par].append(s2t)

            # down-stage of step t-1, emitted after conv(t) so the PE
            # stream never waits on the t-1 LIF3 chain
            pend = None

            def emit_down(t, m3, x_sb):
                out_sb = wpool.tile([128, F], FP32, tag="osb", bufs=2,
                                    name=f"osb_{t}")
                ps_dn = [ppool.tile([128, 2 * CH], FP32, tag="ps",
                                    name=f"psdn_{t}_{p}") for p in range(2)]
                for kt in range(4):       # kt-outer: 4 MMs reuse one LDW
                    for p in range(2):
                        for hh in range(2):
                            c = 2 * p + hh
                            _mm(nc, ps_dn[p][:, hh * CH:(hh + 1) * CH],
                                wdn_sb[:, kt, :],
                                m3[kt][:, c * CH:(c + 1) * CH],
                                start=(kt == 0), stop=False)
                for p in range(2):
                    for hh in range(2):
                        # + bias_dn via rank-2 ones matmul (bf16 hi+lo)
                        _mm(nc, ps_dn[p][:, hh * CH:(hh + 1) * CH],
                            bdn_sb[:], ones_sb[:], start=False, stop=True)
                    nc.vector.tensor_tensor(
                        out=out_sb[:, p * HW:(p + 1) * HW],
                        in0=ps_dn[p][:],
                        in1=x_sb[:, p * HW:(p + 1) * HW],
                        op=ALU.add)
                nc.sync.dma_start(
                    out=o_d[t].rearrange("n c h w -> c n h w"),
                    in_=out_sb.rearrange("p (n h w) -> p n h w", n=NLOC, h=H))

            for t in range(T):
                s2p = s2pp[t % 2]
                # ---- load x_t --------------------------------------------
                x_sb = wpool.tile([128, F], FP32, tag="x", bufs=3,
                                  name=f"x_{t}")
                nc.sync.dma_start(
                    out=x_sb.rearrange("p (n h w) -> p n h w", n=NLOC, h=H),
                    in_=x_d[t].rearrange("n c h w -> c n h w"))

                # ---- LIF1 (bf16, 2^t-scaled like LIF2/3) -----------------
                xs = wpool.tile([128, F], BF16, tag="xs", bufs=1,
                                name=f"xs_{t}")
                nc.vector.tensor_scalar(
                    out=xs[:], in0=x_sb[:], scalar1=float(2 ** t),
                    scalar2=None, op0=ALU.mult)
                if t == 0:
                    w1 = xs
                else:
                    w1 = wpool.tile([128, F], BF16, tag="w1", bufs=1,
                                    name=f"w1_{t}")
                    nc.vector.tensor_tensor(
                        out=w1[:], in0=u1[:], in1=xs[:], op=ALU.add)
                m1 = wpool.tile([128, F], BF16, tag="m1", bufs=1,
                                name=f"m1_{t}")
                nc.vector.tensor_scalar(
                    out=m1[:], in0=w1[:], scalar1=float(2 ** (t + 1)),
                    scalar2=None, op0=ALU.is_lt)
                nc.vector.tensor_tensor(
                    out=u1[:], in0=w1[:], in1=m1[:], op=ALU.mult)

                # ---- down stage of step t-1 (after LIF1 so m1(t) is ready
                # before the PE reaches up(t)) -----------------------------
                if pend is not None:
                    emit_down(*pend)

                # ---- up 1x1 matmul + evac(+bias) + LIF2 ------------------
                h1, w2 = [], []
                for ct in range(4):
                    h1t = wpool.tile([128, F], BF16, tag="hbuf", bufs=6,
                                     name=f"h1_{t}_{ct}")
                    for p in range(2):
                        ps_up = ppool.tile([128, 2 * CH], FP32, tag="ps",
                                           name=f"psup_{t}_{ct}_{p}")
                        for hh in range(2):
                            c = 2 * p + hh
                            _mm(nc, ps_up[:, hh * CH:(hh + 1) * CH],
                                wup_sb[:, 128 * ct:128 * (ct + 1)],
                                m1[:, c * CH:(c + 1) * CH],
                                start=True, stop=True)
                        nc.scalar.activation(
                            out=h1t[:, p * HW:(p + 1) * HW], in_=ps_up[:],
                            func=ACTF.Identity,
                            bias=bia_sb[:, 8 * t + ct:8 * t + ct + 1],
                            scale=float(2 ** t))
                    h1.append(h1t)

                    if t == 0:
                        w2t = h1t
                    else:
                        w2t = wpool.tile([128, F], BF16, tag="wbuf", bufs=5,
                                         name=f"w2_{t}_{ct}")
                        nc.vector.tensor_tensor(
                            out=w2t[:], in0=u2[ct][:], in1=h1t[:], op=ALU.add)
                    w2.append(w2t)
                    nc.vector.tensor_scalar(
                        out=s2p[ct][:, :, 1:1 + H, 2:2 + W],
                        in0=w2t.rearrange("p (n h w) -> p n h w", n=NLOC, h=H),
                        scalar1=float(2 ** (t + 1)), scalar2=None,
                        op0=ALU.is_lt)
                    ueng = nc.vector if ct < 2 else nc.gpsimd
                    ueng.tensor_tensor(
                        out=u2[ct].rearrange("p (n h w) -> p n h w",
                                             n=NLOC, h=H),
                        in0=w2t.rearrange("p (n h w) -> p n h w", n=NLOC, h=H),
                        in1=s2p[ct][:, :, 1:1 + H, 2:2 + W],
                        op=ALU.mult)

                # ---- grouped 3x3 conv (+h1 preload) + evac(+bias) --------
                h2 = [None] * 4
                for q in range(2):
                    ta, tb = 2 * q, 2 * q + 1   # s2 tiles feeding this quad
                    h2a = wpool.tile([128, F], BF16, tag="hbuf", bufs=6,
                                     name=f"h2_{t}_{ta}")
                    h2b = wpool.tile([128, F], BF16, tag="hbuf", bufs=6,
                                     name=f"h2_{t}_{tb}")
                    h2[ta], h2[tb] = h2a, h2b
                    for p in range(2):
                        P1 = ppool.tile([128, 2 * CH], FP32, tag="ps",
                                        name=f"psc1_{t}_{q}_{p}")
                        P2 = ppool.tile([128, 2 * CH], FP32, tag="ps",
                                        name=f"psc2_{t}_{q}_{p}")
                        for hh in range(2):
                            c = 2 * p + hh
                            sl = slice(c * CH, (c + 1) * CH)
                            _mm(nc, P1[:, hh * CH:(hh + 1) * CH],
                                jm_sb[:, 2 * t, :], h1[ta][:, sl],
                                start=True, stop=False)
                        for hh in range(2):
                            c = 2 * p + hh
                            sl = slice(c * CH, (c + 1) * CH)
                            _mm(nc, P2[:, hh * CH:(hh + 1) * CH],
                                jm_sb[:, 2 * t + 1, :], h1[tb][:, sl],
                                start=True, stop=False)
                        for tap in range(9):
                            dy, dx = tap // 3, tap % 3
                            last = tap == 8
                            wq = wcv_sb[:, q, tap, :]
                            for hh in range(2):
                                h0 = 16 * hh
                                osl = slice(hh * CH, (hh + 1) * CH)
                                ra = s2p[ta][:, p, h0 + dy:h0 + dy + 16,
                                             1 + dx:33 + dx]
                                rb = s2p[tb][:, p, h0 + dy:h0 + dy + 16,
                                             1 + dx:33 + dx]
                                # T1: group 4q   rows 0-63  -> P1[0:64]
                                _mm(nc, P1[0:64, osl], wq[0:64, 0:64],
                                    ra[0:64], start=False, stop=last)
                                # T2: group 4q+1 rows 64-127 -> P1[64:128]
                                _mm(nc, P1[64:128, osl], wq[64:128, 64:128],
                                    ra[64:128], start=False, stop=last)
                                # T3: group 4q+2 rows 0-63  -> P2[64:128]
                                _mm(nc, P2[64:128, osl], wq[0:64, 64:128],
                                    rb[0:64], start=False, stop=last)
                                # T4: group 4q+3 rows 64-127 -> P2[0:64]
                                _mm(nc, P2[0:64, osl], wq[64:128, 0:64],
                                    rb[64:128], start=False, stop=last)
                        nc.scalar.activation(
                            out=h2a[:, p * HW:(p + 1) * HW], in_=P1[:],
                            func=ACTF.Identity,
                            bias=bia_sb[:, 8 * t + 4 + ta:8 * t + 5 + ta],
                            scale=float(2 ** t))
                        nc.scalar.activation(
                            out=h2b[:, p * HW:(p + 1) * HW], in_=P2[:],
                            func=ACTF.Identity,
                            bias=bia_sb[:, 8 * t + 4 + tb:8 * t + 5 + tb],
                            scale=float(2 ** t))

                # ---- LIF3 ------------------------------------------------
                m3 = []
                for ct in range(4):
                    if t == 0:
                        w3t = h2[ct]
                    else:
                        w3t = wpool.tile([128, F], BF16, tag="wbuf", bufs=5,
                                         name=f"w3_{t}_{ct}")
                        nc.vector.tensor_tensor(
                            out=w3t[:], in0=u3[ct][:], in1=h2[ct][:],
                            op=ALU.add)
                    m3t = wpool.tile([128, F], BF16, tag="m3", bufs=6,
                                     name=f"m3_{t}_{ct}")
                    nc.vector.tensor_scalar(
                        out=m3t[:], in0=w3t[:],
                        scalar1=float(2 ** (t + 1)), scalar2=None,
                        op0=ALU.is_lt)
                    m3.append(m3t)
                    nc.vector.tensor_tensor(
                        out=u3[ct][:], in0=w3t[:], in1=m3t[:], op=ALU.mult)

                pend = (t, m3, x_sb)

            emit_down(*pend)

    nc.compile()
    return nc


def _prep_weights(inputs):
    """Fold BN into weights, apply the s = 1-2m encoding (scale by -2 and
    compute per-channel ones-biases), pack/permute for the on-chip layout."""
    f32 = np.float32
    sc_up = (inputs["g_up"] / np.sqrt(inputs["v_up"] + EPS)).astype(f32)
    sc_cv = (inputs["g_conv"] / np.sqrt(inputs["v_conv"] + EPS)).astype(f32)
    sc_dn = (inputs["g_down"] / np.sqrt(inputs["v_down"] + EPS)).astype(f32)
    shifts = []
    for nm, sc in (("up", sc_up), ("conv", sc_cv), ("down", sc_dn)):
        shifts.append(inputs[f"b_{nm}"] - inputs[f"m_{nm}"] * sc)
    if max(np.abs(s).max() for s in shifts) > 0:
        raise NotImplementedError("nonzero BN shift not supported")

    w_up = np.asarray(inputs["w_up"], f32)[:, :, 0, 0] * sc_up[:, None]
    wupT = np.ascontiguousarray((-1.0 * w_up).T).astype(BF)    # [128, 512]
    # exact negative sum of the *rounded* weights: the dense m=1
    # background then cancels exactly and only spike terms carry bf16 error
    bias_up = -1.0 * wupT.astype(np.float64).sum(axis=0)       # [512]

    w_cv = np.asarray(inputs["w_conv"], f32) * sc_cv[:, None, None, None]
    wcvP = np.zeros((128, 2, 9, 128), f32)
    for q in range(2):
        for tap in range(9):
            dy, dx = tap // 3, tap % 3

            def blk(g):
                # W_g[ci, co] = -w_conv_eff[64g + co, ci, dy, dx]
                return np.ascontiguousarray(
                    -1.0 * w_cv[64 * g:64 * (g + 1), :, dy, dx].T)
            wcvP[0:64, q, tap, 0:64] = blk(4 * q)
            wcvP[64:128, q, tap, 64:128] = blk(4 * q + 1)
            wcvP[0:64, q, tap, 64:128] = blk(4 * q + 2)
            wcvP[64:128, q, tap, 0:64] = blk(4 * q + 3)
    wcvP = wcvP.astype(BF)
    # conv biases directly per psum partition m (P1 diag / P2 anti-diag),
    # again as exact negative half-sums of the rounded packed weights
    w64 = wcvP.astype(np.float64)
    bias_cv = np.zeros((4, 128))
    for q in range(2):
        lo = w64[0:64, q].sum(axis=(0, 1))     # [128] sum over rows<64, taps
        hi = w64[64:128, q].sum(axis=(0, 1))   # [128] sum over rows>=64
        bias_cv[2 * q] = -1.0 * np.concatenate([lo[:64], hi[64:]])
        bias_cv[2 * q + 1] = -1.0 * np.concatenate([hi[:64], lo[64:]])

    w_dn = np.asarray(inputs["w_down"], f32)[:, :, 0, 0] * sc_dn[:, None]
    # s3/m3 tile layouts: kt even natural, kt odd half-swapped ([g3|g2]...)
    wdnT = np.zeros((128, 4, 128), f32)
    for kt in range(4):
        rows = np.arange(128) + 128 * kt
        if kt % 2 == 1:
            rows = np.concatenate([rows[64:], rows[:64]])
        wdnT[:, kt, :] = -1.0 * w_dn[:, rows].T
    wdnT = wdnT.astype(BF)
    bias_dn = -1.0 * wdnT.astype(np.float64).sum(axis=(0, 1))  # [128]

    # per-t preload matrices: 2^-(t+1) * identity / half-swap (the h1 tiles
    # hold 2^(t+1)-scaled values; the preload rescales them back)
    jm = np.zeros((128, 8, 128), f32)
    for t in range(4):
        sc = 2.0 ** -t
        jm[np.arange(128), 2 * t, np.arange(128)] = sc
        jm[np.arange(128), 2 * t + 1, (np.arange(128) + 64) % 128] = sc
    jm = jm.astype(BF)

    # bias tile [128, 8]: cols 0-3 = up bias per tile; 4-7 = conv bias per
    # conv-out tile (odd tiles half-swapped to match the P2 psum layout)
    # per-t scaled bias tile [128, 8*t + slot]; ACT computes
    # 2^(t+1)*psum + bias so the bias columns carry the same scale
    bias = np.zeros((128, 36), f32)
    for t in range(4):
        bias[:, 32 + t] = 2.0 ** 30 * 2.0 ** (t + 1)
        sc = 2.0 ** t
        for ct in range(4):
            bias[:, 8 * t + ct] = sc * bias_up[128 * ct:128 * (ct + 1)]
            bias[:, 8 * t + 4 + ct] = sc * bias_cv[ct]
    # down bias as bf16 hi + lo rows against a ones vector
    bdn_hi = bias_dn.astype(BF)
    bdn_lo = (bias_dn - bdn_hi.astype(f32)).astype(BF)
    bdn2 = np.stack([bdn_hi, bdn_lo], axis=0)                  # [2, 128]

    return wupT, wcvP, wdnT, jm, bias, bdn2


def run(inputs, trace=False):
    if "nc" not in _CACHE:
        _CACHE["nc"] = _build_nc()
    nc = _CACHE["nc"]

    wupT, wcvP, wdnT, jm, bias, bdn2 = _prep_weights(inputs)
    x = np.asarray(inputs["x"], np.float32)
    in_maps = []
    for i in range(NCORES):
        in_maps.append({
            "x": np.ascontiguousarray(x[:, NLOC * i:NLOC * (i + 1)]),
            "wupT": wupT, "wconvP": wcvP, "wdnT": wdnT, "jmat": jm,
            "bias": bias, "biasdn2": bdn2,
        })
    res = run_bass_kernel_spmd(nc, in_maps, core_ids=list(range(NCORES)),
                               trace=trace)
    out = np.concatenate([r["out"] for r in res.results], axis=1)
    return out, res


def kernel(**inputs):
    out, _ = run(inputs, trace=False)
    return out
